# revision 11
# baseline (speedup 1.0000x reference)
"""Trainium2 Bass kernel for nn_FactorGraphGRU (N=8192, H=64, 8 NeuronCores).

v3 strategy (memory regime): row-shard the output across 8 cores (1024
rows each), then split each core's rows into NBLK=2 column blocks of
512 and stream the contraction dim j column-block-major.  Per block:

  mask phase: 64 bf16 matmuls  psP_b += h(jb)^T-stationary @ posmask
  edge phase: 32+32 fp8 DoubleRow matmuls (K=256 per MM)
              psR_b += hw8 @ relu(eat),  psN_b += hw8 @ min(eat,0)

The relu/min split of the edge adjacency is done on the HOST (same
total DMA bytes as streaming eat + on-chip relu, but saves ~25us/block
of ACT+DVE work and a tile pool).  edge_support is then
es = ap*psR + am*psN with direct PSUM reads.

The per-row post-chain (attention scores, analytic softmax via
Ln/Exp reciprocal, edge GRU, es, node GRU, diag combine) for block b
is emitted under block b's edge phase / block b+1's mask phase, so
only the last block's es+nodeGRU+combine is exposed.

All small parameters ride in TWO packed block DMAs (one f32r for
matmul stationaries, one f32 for biases/rows) so the sync queue isn't
jammed by ~20 small descriptor issues.  The mask stationary h stays
bf16 (fp8 h measured 0.033 rel err, over the 2e-2 gate); the edge
stationary hw is fp8 (no measurable rel-err change).  M (negative
node support) comes from the no-exact-zeros complement
M = (sum_h - h_i) - P.
"""

import numpy as np
from contextlib import ExitStack

N = 8192
H = 64
NCORES = 8
ROWS = N // NCORES        # 1024 output rows per core
JB = 128                  # contraction block (SBUF partitions)
NJB = N // JB             # 64
NPAIR = NJB // 2          # 32
NBLK = 2                  # column blocks per core
BCOLS = ROWS // NBLK      # 512
TJB = 8                   # j-blocks per streamed tile
NT = NJB // TJB           # 8 stream tiles per (block, stream)
ALPHA = 0.2               # leaky relu slope
DEBUG_DUMP = False

# --- packed parameter layouts (shared between host prep and kernel) ---
# f32r pack [128, *]: matmul stationaries + f32r rows
_PR = {}
_c = 0
for _nm, _p, _w in [("weP_rz", 64, 128), ("weM_rz", 64, 128),
                    ("whe_rz", 64, 128), ("wn_rz", 64, 128),
                    ("whn_rz", 64, 128), ("weP_n", 64, 64),
                    ("weM_n", 64, 64), ("whe_hn", 64, 64),
                    ("wn_n", 64, 64), ("whn_hn", 64, 64),
                    ("vaP", 64, 2), ("vaM", 64, 2),
                    ("ones1", 1, 64), ("dn", 1, ROWS), ("de", 1, ROWS),
                    ("hTr", 64, ROWS)]:
    _PR[_nm] = (_p, _c, _w)
    _c += _w
PR_COLS = _c
# f32 pack [128, *]: biases, rows, h
_PF = {}
_c = 0
for _nm, _p, _w in [("be_rz", 128, 1), ("bn_rz", 128, 1),
                    ("be_n", 64, 2), ("bn_n", 64, 2), ("sumh", 64, 1),
                    ("cp", 1, ROWS), ("cn", 1, ROWS), ("hTf", 64, ROWS)]:
    _PF[_nm] = (_p, _c, _w)
    _c += _w
PF_COLS = _c


# ---------------------------------------------------------------------------
# walrus workaround: this toolchain accepts at most ONE sync wait per
# instruction; Tile attaches several.  Rewrite the BIR so every extra wait
# rides on its own NoOp carrier right before the instruction.
# ---------------------------------------------------------------------------
def _split_multiwaits(nc):
    import bass_rust
    import concourse.mybir as mybir

    ctr = [0]

    def carrier(engine, wait):
        ctr[0] += 1
        nop = bass_rust.InstNoOp(name=f"WS-{ctr[0]}", engine=engine, ins=[], outs=[])
        nop.sync_info = mybir.SyncInfo(on_wait=[wait], on_update=[])
        return nop

    for fn in nc.m.functions:
        stack = list(fn.blocks)
        while stack:
            bb = stack.pop()
            stack.extend(getattr(bb, "blocks", []) or [])
            out = []
            changed = False
            for inst in bb.instructions:
                si = inst.sync_info
                waits = list(si.on_wait) if si is not None and si.on_wait else []
                if len(waits) > 1:
                    for w in waits[:-1]:
                        out.append(carrier(inst.engine, w))
                    si.on_wait = [waits[-1]]
                    changed = True
                out.append(inst)
            if changed:
                bb.instructions = out


def _build_nc():
    import concourse.bass as bass
    import concourse.tile as tile
    from concourse import mybir

    F32 = mybir.dt.float32
    F32R = mybir.dt.float32r
    BF16 = mybir.dt.bfloat16
    F8 = mybir.dt.float8e4
    AF = mybir.ActivationFunctionType
    OP = mybir.AluOpType
    DR = mybir.MatmulPerfMode.DoubleRow

    nc = bass.Bass("TRN2", target_bir_lowering=False, debug=False,
                   num_devices=NCORES)

    msk_d = nc.dram_tensor("msk", [NBLK * NT * JB, TJB, BCOLS], F8,
                           kind="ExternalInput").ap()
    eatR_d = nc.dram_tensor("eatR", [NBLK * NT * JB, TJB, BCOLS], F8,
                            kind="ExternalInput").ap()
    eatN_d = nc.dram_tensor("eatN", [NBLK * NT * JB, TJB, BCOLS], F8,
                            kind="ExternalInput").ap()
    hb_d = nc.dram_tensor("hb", [JB, NJB * H], BF16, kind="ExternalInput").ap()
    hw8_d = nc.dram_tensor("hw8", [JB, NJB, H], F8, kind="ExternalInput").ap()
    pr_d = nc.dram_tensor("pr", [JB, PR_COLS], F32R, kind="ExternalInput").ap()
    pf_d = nc.dram_tensor("pf", [JB, PF_COLS], F32, kind="ExternalInput").ap()
    out = nc.dram_tensor("out", [H, ROWS], F32, kind="ExternalOutput").ap()
    dbg = {}
    if DEBUG_DUMP:
        for nm, sh in [("d_xp", [H, ROWS]), ("d_xm", [H, ROWS]),
                       ("d_ep", [1, ROWS]), ("d_em", [1, ROWS]),
                       ("d_ap", [1, ROWS]), ("d_am", [1, ROWS]),
                       ("d_es", [H, ROWS]), ("d_eo", [H, ROWS]),
                       ("d_no", [H, ROWS])]:
            dbg[nm] = nc.dram_tensor(nm, sh, F32, kind="ExternalOutput").ap()

    with tile.TileContext(nc) as tc, ExitStack() as ctx:
        stat = ctx.enter_context(tc.tile_pool(name="stat", bufs=1))
        adj = ctx.enter_context(tc.tile_pool(name="adj", bufs=1))
        work = ctx.enter_context(tc.tile_pool(name="work", bufs=1))
        psS = ctx.enter_context(tc.tile_pool(name="psS", bufs=1, space="PSUM"))
        psG = ctx.enter_context(tc.tile_pool(name="psG", bufs=2, space="PSUM"))

        st = {}

        def pr(nm):
            p, c, w = _PR[nm]
            return st["pr"][0:p, c:c + w]

        def pf(nm):
            p, c, w = _PF[nm]
            return st["pf"][0:p, c:c + w]

        # --- streamed tiles ---
        mskt_tiles = {}
        etR_tiles = {}
        etN_tiles = {}

        def dma_mask(g):
            t = adj.tile([JB, TJB, BCOLS], F8, name="mskt", tag="mskt", bufs=3)
            nc.sync.dma_start(t[:], msk_d[g * JB:(g + 1) * JB, :, :])
            mskt_tiles[g] = t

        def dma_edge(g):
            # edge streams ride the gpsimd DMA queue so a ring-buffer wait
            # can never head-of-line-block the mask stream (sync queue)
            t = adj.tile([JB, TJB, BCOLS], F8, name="etR", tag="etR", bufs=3)
            nc.gpsimd.dma_start(t[:], eatR_d[g * JB:(g + 1) * JB, :, :])
            etR_tiles[g] = t
            t = adj.tile([JB, TJB, BCOLS], F8, name="etN", tag="etN", bufs=3)
            nc.gpsimd.dma_start(t[:], eatN_d[g * JB:(g + 1) * JB, :, :])
            etN_tiles[g] = t

        # --- accumulators: 2 bufs per tag + psG(2) = 8 PSUM banks ---
        def psum_blk(tag, b):
            key = f"{tag}{b}"
            if key not in st:
                st[key] = psS.tile([H, BCOLS], F32, name=key, tag=tag, bufs=2)
            return st[key]

        def emit_mask_tile(b, q):
            t = mskt_tiles.pop(b * NT + q)
            psP = psum_blk("psP", b)
            for s in range(TJB):
                jb = q * TJB + s
                hbt = st[f"hb{jb // 16}"]
                nc.tensor.matmul(psP[:], hbt[:, (jb % 16) * H:(jb % 16 + 1) * H],
                                 t[:, s, :],
                                 start=(jb == 0), stop=(jb == NJB - 1))

        def emit_edge_tile(b, q):
            tR = etR_tiles.pop(b * NT + q)
            tN = etN_tiles.pop(b * NT + q)
            hw8 = st["hw8"]
            psR = psum_blk("psR", b)
            psN = psum_blk("psN", b)
            for k in range(TJB // 2):
                jb = q * TJB + 2 * k
                stt = hw8[:, jb:jb + 2, :]
                first = (jb == 0)
                last = (jb == NJB - 2)
                nc.tensor.matmul(psR[:], stt, tR[:, 2 * k:2 * k + 2, :],
                                 start=first, stop=last, perf_mode=DR)
                nc.tensor.matmul(psN[:], stt, tN[:, 2 * k:2 * k + 2, :],
                                 start=first, stop=last, perf_mode=DR)

        # --- per-block post work, emitted as thunks under later stream ---
        def cs_of(b):
            return slice(b * BCOLS, (b + 1) * BCOLS)

        def row_t(name, b, dt=F32):
            t = work.tile([1, BCOLS], dt, name=f"{name}{b}", tag=name, bufs=1)
            st[f"{name}{b}"] = t
            return t

        def th_xpxm(b):
            cs = cs_of(b)
            xp = work.tile([H, BCOLS], F32R, name=f"xp{b}", tag="xp", bufs=1)
            st[f"xp{b}"] = xp
            nc.scalar.copy(xp[:], psum_blk("psP", b)[:])
            xm = work.tile([H, BCOLS], F32R, name=f"xm{b}", tag="xm", bufs=1)
            st[f"xm{b}"] = xm
            # xm = (h - sum_h) + P   (== -M)
            nc.vector.scalar_tensor_tensor(
                xm[:], pf("hTf")[:, cs], pf("sumh"),
                xp[:].bitcast(F32), OP.subtract, OP.add)

        def th_scores(b):
            xp, xm = st[f"xp{b}"], st[f"xm{b}"]
            for col, nm in ((0, "ep"), (1, "em")):
                g = psG.tile([1, BCOLS], F32, name=f"{nm}{b}_ps", tag="g")
                nc.tensor.matmul(g[:], pr("vaP")[:, col:col + 1], xp[:],
                                 start=True, stop=False)
                nc.tensor.matmul(g[:], pr("vaM")[:, col:col + 1], xm[:],
                                 start=False, stop=True)
                dst = row_t(nm, b)
                nc.scalar.activation(dst[:], g[:], AF.Lrelu, alpha=ALPHA)

        def th_softmax1(b):
            ep, em = st[f"ep{b}"], st[f"em{b}"]
            m = row_t("mrow", b)
            nc.vector.tensor_tensor(m[:], ep[:], em[:], OP.max)
            wp = row_t("wp", b)
            nc.vector.tensor_tensor(wp[:], ep[:], m[:], OP.subtract)
            nc.scalar.activation(wp[:], wp[:], AF.Exp)
            wm = row_t("wm", b)
            nc.vector.tensor_tensor(wm[:], em[:], m[:], OP.subtract)
            nc.scalar.activation(wm[:], wm[:], AF.Exp)

        def th_softmax2(b):
            cs = cs_of(b)
            wp, wm = st[f"wp{b}"], st[f"wm{b}"]
            z = row_t("zrow", b)
            nc.vector.tensor_tensor(z[:], pf("cp")[:, cs], wp[:], OP.mult)
            tz = row_t("tz", b)
            nc.vector.tensor_tensor(tz[:], pf("cn")[:, cs], wm[:], OP.mult)
            nc.vector.tensor_tensor(z[:], z[:], tz[:], OP.add)
            # 1/z via exp(-ln z): ACT table accuracy is plenty for the
            # softmax normalizer, and it keeps the slow DVE reciprocal off
            # the critical path
            iz = row_t("iz", b)
            nc.scalar.activation(iz[:], z[:], AF.Ln)
            nc.scalar.activation(iz[:], iz[:], AF.Exp, scale=-1.0)
            ap = row_t("a_p", b, F32R)
            nc.vector.tensor_tensor(ap[:], wp[:], iz[:], OP.mult)
            am = row_t("a_m", b, F32R)
            nc.vector.tensor_tensor(am[:], wm[:], iz[:], OP.mult)

        def bcast(src_r, b, name):
            g = psG.tile([H, BCOLS], F32, name=f"{name}{b}_ps", tag="g")
            nc.tensor.matmul(g[:], pr("ones1"), src_r, start=True, stop=True)
            t = work.tile([H, BCOLS], F32, name=f"{name}{b}", tag=name, bufs=1)
            st[f"{name}{b}"] = t
            nc.scalar.copy(t[:], g[:])
            return t

        def th_bcasts(b):
            cs = cs_of(b)
            bcast(st[f"a_p{b}"][:], b, "ap_b")
            bcast(st[f"a_m{b}"][:], b, "am_b")
            bcast(pr("de")[:, cs], b, "de_b")
            bcast(pr("dn")[:, cs], b, "dn_b")

        # --- GRU pieces (transposed layout [feat, col]) ---
        def gru_gates(b, xs, w_rz, w_n, whh_rz, whh_hn, b_rz, b_n2, pref):
            hTr = pr("hTr")[:, cs_of(b)]
            g_rz = psG.tile([2 * H, BCOLS], F32, name=f"{pref}rz{b}", tag="g")
            mms = [(pr(w), mv) for mv, w in zip(xs, w_rz)]
            mms.append((pr(whh_rz), hTr))
            for i, (lh, mv) in enumerate(mms):
                nc.tensor.matmul(g_rz[:], lh, mv, start=(i == 0),
                                 stop=(i == len(mms) - 1))
            rz = work.tile([2 * H, BCOLS], F32, name=f"{pref}rzt{b}",
                           tag=f"{pref}rzt", bufs=1)
            st[f"{pref}rz{b}"] = rz
            nc.scalar.activation(rz[:], g_rz[:], AF.Sigmoid, bias=pf(b_rz))
            # unpack z to partition 0 (tensor_tensor requires same start
            # partition on all operands; a shifted COPY is legal)
            z_t = work.tile([H, BCOLS], F32, name=f"{pref}z{b}",
                            tag=f"g{pref}z", bufs=1)
            st[f"{pref}z{b}"] = z_t
            nc.vector.tensor_copy(z_t[:], rz[H:2 * H, :])

            g_n = psG.tile([H, BCOLS], F32, name=f"{pref}n{b}", tag="g")
            for i, (mv, w) in enumerate(zip(xs, w_n)):
                nc.tensor.matmul(g_n[:], pr(w), mv, start=(i == 0),
                                 stop=(i == len(xs) - 1))
            ns = work.tile([H, BCOLS], F32, name=f"{pref}ns{b}",
                           tag=f"{pref}ns", bufs=1)
            st[f"{pref}ns{b}"] = ns
            nc.scalar.activation(ns[:], g_n[:], AF.Identity,
                                 bias=pf(b_n2)[:, 0:1])

            g_h = psG.tile([H, BCOLS], F32, name=f"{pref}hn{b}", tag="g")
            nc.tensor.matmul(g_h[:], pr(whh_hn), hTr, start=True, stop=True)
            hn = work.tile([H, BCOLS], F32, name=f"{pref}hnt{b}",
                           tag=f"{pref}hnt", bufs=1)
            st[f"{pref}hn{b}"] = hn
            nc.scalar.activation(hn[:], g_h[:], AF.Identity,
                                 bias=pf(b_n2)[:, 1:2])

        def gru_finish(b, pref, c0=0, c1=BCOLS):
            """n = tanh(ns + r*hn); out = n + z*(h - n), over cols [c0:c1)"""
            cs = slice(b * BCOLS + c0, b * BCOLS + c1)
            hsl = slice(c0, c1)
            rz, ns, hn = st[f"{pref}rz{b}"], st[f"{pref}ns{b}"], st[f"{pref}hn{b}"]
            key = f"{pref}o{b}"
            if key not in st:
                st[key] = work.tile([H, BCOLS], F32, name=key, tag=f"g{pref}o",
                                    bufs=1)
                st[f"{pref}ng{b}"] = work.tile([H, BCOLS], F32,
                                               name=f"{pref}ng{b}",
                                               tag=f"g{pref}n", bufs=1)
                st[f"{pref}dd{b}"] = work.tile([H, BCOLS], F32,
                                               name=f"{pref}dd{b}",
                                               tag=f"g{pref}d", bufs=1)
            og, n_g, d = st[key], st[f"{pref}ng{b}"], st[f"{pref}dd{b}"]
            nc.vector.tensor_tensor(d[:, hsl], hn[:, hsl], rz[0:H, hsl],
                                    OP.mult)
            nc.gpsimd.tensor_tensor(ns[:, hsl], ns[:, hsl], d[:, hsl], OP.add)
            nc.scalar.activation(n_g[:, hsl], ns[:, hsl], AF.Tanh)
            nc.vector.tensor_tensor(d[:, hsl], pf("hTf")[:, cs], n_g[:, hsl],
                                    OP.subtract)
            nc.vector.tensor_tensor(og[:, hsl], d[:, hsl],
                                    st[f"{pref}z{b}"][:, hsl], OP.mult)
            nc.gpsimd.tensor_tensor(og[:, hsl], og[:, hsl], n_g[:, hsl],
                                    OP.add)
            return og

        def th_egru_gates(b):
            gru_gates(b, [st[f"xp{b}"][:], st[f"xm{b}"][:]],
                      ["weP_rz", "weM_rz"], ["weP_n", "weM_n"],
                      "whe_rz", "whe_hn", "be_rz", "be_n", "e")

        def th_egru_finish(b):
            st[f"eo{b}"] = gru_finish(b, "e")

        def th_fin_e(b):
            fin = work.tile([H, BCOLS], F32, name=f"fin{b}", tag="fin", bufs=1)
            st[f"fin{b}"] = fin
            nc.vector.tensor_tensor(fin[:], st[f"de_b{b}"][:], st[f"eo{b}"][:],
                                    OP.mult)

        def th_es(b):
            # es = ap*S+ + am*S-  with S+ = psR, S- = psN (PSUM reads)
            t1 = work.tile([H, BCOLS], F32, name=f"t1{b}", tag="t1", bufs=1)
            nc.vector.tensor_tensor(t1[:], st[f"ap_b{b}"][:],
                                    psum_blk("psR", b)[:], OP.mult)
            t2 = work.tile([H, BCOLS], F32, name=f"t2{b}", tag="t2", bufs=1)
            nc.scalar.copy(t2[:], psum_blk("psN", b)[:])
            t3 = work.tile([H, BCOLS], F32, name=f"t3{b}", tag="t3", bufs=1)
            nc.vector.tensor_tensor(t3[:], st[f"am_b{b}"][:], t2[:], OP.mult)
            es = work.tile([H, BCOLS], F32R, name=f"es{b}", tag="es", bufs=1)
            st[f"es{b}"] = es
            nc.vector.tensor_tensor(es[:], t1[:], t3[:], OP.add)

        def th_ngru_gates(b):
            gru_gates(b, [st[f"es{b}"][:]], ["wn_rz"], ["wn_n"],
                      "whn_rz", "whn_hn", "bn_rz", "bn_n", "n")

        def th_combine(b, c0, c1):
            cs = slice(b * BCOLS + c0, b * BCOLS + c1)
            hsl = slice(c0, c1)
            no = gru_finish(b, "n", c0, c1)
            key = f"tf{b}"
            if key not in st:
                st[key] = work.tile([H, BCOLS], F32, name=key, tag="tf",
                                    bufs=1)
                st[f"ot{b}"] = work.tile([H, BCOLS], F32, name=f"ot{b}",
                                         tag="ot", bufs=1)
            tf, ot = st[key], st[f"ot{b}"]
            nc.vector.tensor_tensor(tf[:, hsl], st[f"dn_b{b}"][:, hsl],
                                    no[:, hsl], OP.mult)
            nc.gpsimd.tensor_tensor(ot[:, hsl], st[f"fin{b}"][:, hsl],
                                    tf[:, hsl], OP.add)
            nc.sync.dma_start(out[:, cs], ot[:, hsl])
            if DEBUG_DUMP and c1 == BCOLS:
                csf = cs_of(b)
                for nm, key2 in [("d_xp", f"xp{b}"), ("d_xm", f"xm{b}"),
                                 ("d_es", f"es{b}"), ("d_eo", f"eo{b}"),
                                 ("d_no", f"no{b}")]:
                    if key2 in st:
                        nc.sync.dma_start(dbg[nm][:, csf],
                                          st[key2][:].bitcast(F32))
                for nm, key2 in [("d_ep", f"ep{b}"), ("d_em", f"em{b}"),
                                 ("d_ap", f"a_p{b}"), ("d_am", f"a_m{b}")]:
                    nc.sync.dma_start(dbg[nm][:, csf],
                                      st[key2][:].bitcast(F32))

        # ================= schedule =================
        NSLOT = NBLK * 2 * NT
        sched = {}

        def at(slot, th):
            sched.setdefault(slot, []).append(th)

        for b in range(NBLK):
            s0 = b * 2 * NT
            at(s0 + NT - 1, lambda b=b: th_xpxm(b))
            at(s0 + NT + 0, lambda b=b: th_scores(b))
            at(s0 + NT + 1, lambda b=b: th_softmax1(b))
            at(s0 + NT + 2, lambda b=b: th_softmax2(b))
            at(s0 + NT + 3, lambda b=b: th_bcasts(b))
            at(s0 + NT + 4, lambda b=b: th_egru_gates(b))
            at(s0 + NT + 6, lambda b=b: th_egru_finish(b))
            at(s0 + NT + 7, lambda b=b: th_fin_e(b))
            at(s0 + 2 * NT + 0, lambda b=b: th_es(b))
            at(s0 + 2 * NT + 1, lambda b=b: th_ngru_gates(b))
            at(s0 + 2 * NT + 3, lambda b=b: th_combine(b, 0, 256))
            at(s0 + 2 * NT + 4, lambda b=b: th_combine(b, 256, BCOLS))

        # initial DMAs: first mask tile + first stationary quarter ASAP
        dma_mask(0)
        QW = 16 * H
        for i in range(4):
            t = stat.tile([JB, QW], BF16, name=f"hb{i}")
            st[f"hb{i}"] = t
            nc.sync.dma_start(t[:], hb_d[:, i * QW:(i + 1) * QW])
            if i == 0:
                dma_mask(1)
        hw8_t = stat.tile([JB, NJB, H], F8, name="hw8")
        st["hw8"] = hw8_t
        nc.sync.dma_start(hw8_t[:], hw8_d[:])
        dma_edge(0)
        st["pr"] = stat.tile([JB, PR_COLS], F32R, name="pr")
        nc.sync.dma_start(st["pr"][:], pr_d[:])
        st["pf"] = stat.tile([JB, PF_COLS], F32, name="pf")
        nc.sync.dma_start(st["pf"][:], pf_d[:])

        mask_issued = 2
        edge_issued = 1

        for s in range(NSLOT):
            b, phase = divmod(s, 2 * NT)
            while mask_issued < NBLK * NT:
                g = mask_issued
                cons = (g // NT) * 2 * NT + (g % NT)
                if cons > s + 2:
                    break
                dma_mask(g)
                mask_issued += 1
            while edge_issued < NBLK * NT:
                g = edge_issued
                cons = (g // NT) * 2 * NT + NT + (g % NT)
                if cons > s + 2:
                    break
                dma_edge(g)
                edge_issued += 1
            if phase < NT:
                emit_mask_tile(b, phase)
            else:
                emit_edge_tile(b, phase - NT)
            for th in sched.pop(s, []):
                th()
        for s in sorted(sched):
            for th in sched[s]:
                th()

    _split_multiwaits(nc)
    return nc


def _host_prep(inputs):
    import ml_dtypes
    BF = ml_dtypes.bfloat16
    F8 = ml_dtypes.float8_e4m3

    h = np.ascontiguousarray(inputs["h"], dtype=np.float32)
    node_adj = inputs["node_adj"]
    edge_adj = inputs["edge_adj"]
    W_gat = np.asarray(inputs["W_gat"], dtype=np.float32)
    a_gat = np.asarray(inputs["a_gat"], dtype=np.float32)
    w_ih_e = np.asarray(inputs["w_ih_e"], dtype=np.float32)
    w_hh_e = np.asarray(inputs["w_hh_e"], dtype=np.float32)
    b_ih_e = np.asarray(inputs["b_ih_e"], dtype=np.float32)
    b_hh_e = np.asarray(inputs["b_hh_e"], dtype=np.float32)
    w_ih_n = np.asarray(inputs["w_ih_n"], dtype=np.float32)
    w_hh_n = np.asarray(inputs["w_hh_n"], dtype=np.float32)
    b_ih_n = np.asarray(inputs["b_ih_n"], dtype=np.float32)
    b_hh_n = np.asarray(inputs["b_hh_n"], dtype=np.float32)

    d_node = np.ascontiguousarray(np.diag(node_adj)).astype(np.float32)
    d_edge = np.ascontiguousarray(np.diag(edge_adj)).astype(np.float32)

    idx = np.arange(N)
    posn = (node_adj.T > 0).astype(F8)
    posn[idx, idx] = F8(0)
    eat = edge_adj.T.astype(F8).astype(np.float32)
    eat[idx, idx] = 0.0
    eatR = np.maximum(eat, 0.0).astype(F8)
    eatN = np.minimum(eat, 0.0).astype(F8)
    cp_full = ((edge_adj > 0).sum(axis=1)
               - (np.diag(edge_adj) > 0)).astype(np.float32)
    cn_full = (N - 1) - cp_full

    hw = (h @ W_gat).astype(np.float32)
    hb = np.ascontiguousarray(
        h.reshape(NJB, JB, H).transpose(1, 0, 2).reshape(JB, NJB * H)
    ).astype(BF)
    # edge stationary, pair-interleaved for DoubleRow:
    # hw8[p, 2q+s, m] = hw[q*256 + s*128 + p, m]
    hw8 = np.ascontiguousarray(
        hw.reshape(NPAIR, 2, JB, H).transpose(2, 0, 1, 3).reshape(JB, NJB, H)
    ).astype(F8)
    sum_h = h.sum(axis=0, dtype=np.float64).astype(np.float32)

    a1 = a_gat[0:H, 0]
    a2 = a_gat[H:2 * H, 0]
    wih_eT = w_ih_e.T
    wihn_T = w_ih_n.T
    vals = {
        "weP_rz": wih_eT[0:H, 0:2 * H], "weM_rz": -wih_eT[H:2 * H, 0:2 * H],
        "whe_rz": w_hh_e.T[:, 0:2 * H], "wn_rz": wihn_T[:, 0:2 * H],
        "whn_rz": w_hh_n.T[:, 0:2 * H],
        "weP_n": wih_eT[0:H, 2 * H:], "weM_n": -wih_eT[H:2 * H, 2 * H:],
        "whe_hn": w_hh_e.T[:, 2 * H:], "wn_n": wihn_T[:, 2 * H:],
        "whn_hn": w_hh_n.T[:, 2 * H:],
        "vaP": np.stack([W_gat @ a1, W_gat @ a2], axis=1),
        "vaM": np.stack([-(W_gat @ a2), -(W_gat @ a1)], axis=1),
        "ones1": np.ones((1, H)),
        "be_rz": (b_ih_e + b_hh_e)[0:2 * H].reshape(2 * H, 1),
        "bn_rz": (b_ih_n + b_hh_n)[0:2 * H].reshape(2 * H, 1),
        "be_n": np.stack([b_ih_e[2 * H:], b_hh_e[2 * H:]], 1),
        "bn_n": np.stack([b_ih_n[2 * H:], b_hh_n[2 * H:]], 1),
        "sumh": sum_h.reshape(H, 1),
    }

    def pack(layout, per_core):
        cols = PR_COLS if layout is _PR else PF_COLS
        arr = np.zeros((JB, cols), np.float32)
        for nm, (p, c, w) in layout.items():
            v = per_core.get(nm)
            if v is None:
                v = vals[nm]
            arr[0:p, c:c + w] = v
        return arr

    def stream_pack(x):
        y = x.reshape(NT, TJB, JB, NBLK, BCOLS)
        return np.ascontiguousarray(
            y.transpose(3, 0, 2, 1, 4).reshape(NBLK * NT * JB, TJB, BCOLS))

    in_maps = []
    for c in range(NCORES):
        sl = slice(c * ROWS, (c + 1) * ROWS)
        hTc = np.ascontiguousarray(h[sl].T)
        per_r = {"dn": d_node[sl].reshape(1, ROWS),
                 "de": d_edge[sl].reshape(1, ROWS), "hTr": hTc}
        per_f = {"cp": cp_full[sl].reshape(1, ROWS),
                 "cn": cn_full[sl].reshape(1, ROWS), "hTf": hTc}
        m = {
            "msk": stream_pack(posn[:, sl]),
            "eatR": stream_pack(eatR[:, sl]),
            "eatN": stream_pack(eatN[:, sl]),
            "hb": hb, "hw8": hw8,
            "pr": pack(_PR, per_r),
            "pf": pack(_PF, per_f),
        }
        in_maps.append(m)
    return in_maps


def _run(inputs, trace=False, tmpdir=None):
    from concourse.bass_utils import run_bass_kernel_spmd

    in_maps = _host_prep(inputs)
    nc = _build_nc()
    res = run_bass_kernel_spmd(nc, in_maps, core_ids=list(range(NCORES)),
                               trace=trace, tmpdir=tmpdir)
    outs = [res.results[c]["out"] for c in range(NCORES)]       # [64, 1024] each
    full = np.concatenate([o.T for o in outs], axis=0)          # [8192, 64]
    return np.ascontiguousarray(full, dtype=np.float32), res


def kernel(**inputs):
    out, _ = _run(inputs, trace=False)
    return out


# revision 12
# speedup vs baseline: 1.0366x; 1.0366x over previous
"""Trainium2 Bass kernel for nn_FactorGraphGRU (N=8192, H=64, 8 NeuronCores).

v3 strategy (memory regime): row-shard the output across 8 cores (1024
rows each), then split each core's rows into NBLK=2 column blocks of
512 and stream the contraction dim j column-block-major.  Per block:

  mask phase: 64 bf16 matmuls  psP_b += h(jb)^T-stationary @ posmask
  edge phase: 32+32 fp8 DoubleRow matmuls (K=256 per MM)
              psR_b += hw8 @ relu(eat),  psN_b += hw8 @ min(eat,0)

The relu/min split of the edge adjacency is done on the HOST (same
total DMA bytes as streaming eat + on-chip relu, but saves ~25us/block
of ACT+DVE work and a tile pool).  edge_support is then
es = ap*psR + am*psN with direct PSUM reads.

The per-row post-chain (attention scores, analytic softmax via
Ln/Exp reciprocal, edge GRU, es, node GRU, diag combine) for block b
is emitted under block b's edge phase / block b+1's mask phase, so
only the last block's es+nodeGRU+combine is exposed.

All small parameters ride in TWO packed block DMAs (one f32r for
matmul stationaries, one f32 for biases/rows) so the sync queue isn't
jammed by ~20 small descriptor issues.  The mask stationary h stays
bf16 (fp8 h measured 0.033 rel err, over the 2e-2 gate); the edge
stationary hw is fp8 (no measurable rel-err change).  M (negative
node support) comes from the no-exact-zeros complement
M = (sum_h - h_i) - P.
"""

import numpy as np
from contextlib import ExitStack

N = 8192
H = 64
NCORES = 8
ROWS = N // NCORES        # 1024 output rows per core
JB = 128                  # contraction block (SBUF partitions)
NJB = N // JB             # 64
NPAIR = NJB // 2          # 32
NBLK = 2                  # column blocks per core
BCOLS = ROWS // NBLK      # 512
TJB = 8                   # j-blocks per streamed tile
NT = NJB // TJB           # 8 stream tiles per (block, stream)
ALPHA = 0.2               # leaky relu slope
DEBUG_DUMP = False

# --- packed parameter layouts (shared between host prep and kernel) ---
# f32r pack [128, *]: matmul stationaries + f32r rows
_PR = {}
_c = 0
for _nm, _p, _w in [("weP_rz", 64, 128), ("weM_rz", 64, 128),
                    ("whe_rz", 64, 128), ("wn_rz", 64, 128),
                    ("whn_rz", 64, 128), ("weP_n", 64, 64),
                    ("weM_n", 64, 64), ("whe_hn", 64, 64),
                    ("wn_n", 64, 64), ("whn_hn", 64, 64),
                    ("vaP", 64, 2), ("vaM", 64, 2),
                    ("ones1", 1, 64), ("dn", 1, ROWS), ("de", 1, ROWS),
                    ("hTr", 64, ROWS)]:
    _PR[_nm] = (_p, _c, _w)
    _c += _w
PR_COLS = _c
# f32 pack [128, *]: biases, rows, h
_PF = {}
_c = 0
for _nm, _p, _w in [("be_rz", 128, 1), ("bn_rz", 128, 1),
                    ("be_n", 64, 2), ("bn_n", 64, 2), ("sumh", 64, 1),
                    ("cp", 1, ROWS), ("cn", 1, ROWS), ("hTf", 64, ROWS)]:
    _PF[_nm] = (_p, _c, _w)
    _c += _w
PF_COLS = _c


# ---------------------------------------------------------------------------
# walrus workaround: this toolchain accepts at most ONE sync wait per
# instruction; Tile attaches several.  Rewrite the BIR so every extra wait
# rides on its own NoOp carrier right before the instruction.
# ---------------------------------------------------------------------------
def _split_multiwaits(nc):
    import bass_rust
    import concourse.mybir as mybir

    ctr = [0]

    def carrier(engine, wait):
        ctr[0] += 1
        nop = bass_rust.InstNoOp(name=f"WS-{ctr[0]}", engine=engine, ins=[], outs=[])
        nop.sync_info = mybir.SyncInfo(on_wait=[wait], on_update=[])
        return nop

    for fn in nc.m.functions:
        stack = list(fn.blocks)
        while stack:
            bb = stack.pop()
            stack.extend(getattr(bb, "blocks", []) or [])
            out = []
            changed = False
            for inst in bb.instructions:
                si = inst.sync_info
                waits = list(si.on_wait) if si is not None and si.on_wait else []
                if len(waits) > 1:
                    for w in waits[:-1]:
                        out.append(carrier(inst.engine, w))
                    si.on_wait = [waits[-1]]
                    changed = True
                out.append(inst)
            if changed:
                bb.instructions = out


def _build_nc():
    import concourse.bass as bass
    import concourse.tile as tile
    from concourse import mybir

    F32 = mybir.dt.float32
    F32R = mybir.dt.float32r
    BF16 = mybir.dt.bfloat16
    F8 = mybir.dt.float8e4
    AF = mybir.ActivationFunctionType
    OP = mybir.AluOpType
    DR = mybir.MatmulPerfMode.DoubleRow

    nc = bass.Bass("TRN2", target_bir_lowering=False, debug=False,
                   num_devices=NCORES)

    msk_d = nc.dram_tensor("msk", [NBLK * NT * JB, TJB, BCOLS], F8,
                           kind="ExternalInput").ap()
    eat_d = nc.dram_tensor("eat", [NBLK * NT * JB, TJB, BCOLS], F8,
                           kind="ExternalInput").ap()
    hb_d = nc.dram_tensor("hb", [JB, NJB * H], BF16, kind="ExternalInput").ap()
    hw8_d = nc.dram_tensor("hw8", [JB, NJB, H], F8, kind="ExternalInput").ap()
    pr_d = nc.dram_tensor("pr", [JB, PR_COLS], F32R, kind="ExternalInput").ap()
    pf_d = nc.dram_tensor("pf", [JB, PF_COLS], F32, kind="ExternalInput").ap()
    out = nc.dram_tensor("out", [H, ROWS], F32, kind="ExternalOutput").ap()
    dbg = {}
    if DEBUG_DUMP:
        for nm, sh in [("d_xp", [H, ROWS]), ("d_xm", [H, ROWS]),
                       ("d_ep", [1, ROWS]), ("d_em", [1, ROWS]),
                       ("d_ap", [1, ROWS]), ("d_am", [1, ROWS]),
                       ("d_es", [H, ROWS]), ("d_eo", [H, ROWS]),
                       ("d_no", [H, ROWS])]:
            dbg[nm] = nc.dram_tensor(nm, sh, F32, kind="ExternalOutput").ap()

    with tile.TileContext(nc) as tc, ExitStack() as ctx:
        stat = ctx.enter_context(tc.tile_pool(name="stat", bufs=1))
        adj = ctx.enter_context(tc.tile_pool(name="adj", bufs=1))
        work = ctx.enter_context(tc.tile_pool(name="work", bufs=1))
        psS = ctx.enter_context(tc.tile_pool(name="psS", bufs=1, space="PSUM"))
        psG = ctx.enter_context(tc.tile_pool(name="psG", bufs=2, space="PSUM"))

        st = {}

        def pr(nm):
            p, c, w = _PR[nm]
            return st["pr"][0:p, c:c + w]

        def pf(nm):
            p, c, w = _PF[nm]
            return st["pf"][0:p, c:c + w]

        # --- streamed tiles ---
        mskt_tiles = {}
        et_tiles = {}

        def dma_mask(g):
            t = adj.tile([JB, TJB, BCOLS], F8, name="mskt", tag="mskt", bufs=3)
            nc.sync.dma_start(t[:], msk_d[g * JB:(g + 1) * JB, :, :])
            mskt_tiles[g] = t

        def dma_edge(g):
            # edge stream rides the gpsimd DMA queue so a ring-buffer wait
            # can never head-of-line-block the mask stream (sync queue)
            t = adj.tile([JB, TJB, BCOLS], F8, name="et", tag="et", bufs=3)
            nc.gpsimd.dma_start(t[:], eat_d[g * JB:(g + 1) * JB, :, :])
            et_tiles[g] = t

        # --- accumulators: 2 bufs per tag + psG(2) = 8 PSUM banks ---
        def psum_blk(tag, b):
            key = f"{tag}{b}"
            if key not in st:
                st[key] = psS.tile([H, BCOLS], F32, name=key, tag=tag, bufs=2)
            return st[key]

        def emit_mask_tile(b, q):
            t = mskt_tiles.pop(b * NT + q)
            psP = psum_blk("psP", b)
            for s in range(TJB):
                jb = q * TJB + s
                hbt = st[f"hb{jb // 16}"]
                nc.tensor.matmul(psP[:], hbt[:, (jb % 16) * H:(jb % 16 + 1) * H],
                                 t[:, s, :],
                                 start=(jb == 0), stop=(jb == NJB - 1))

        ACT_SEGS = 3   # relu segments on ACT; rest on DVE (faster for fp8)

        def emit_edge_tile(b, q):
            t = et_tiles.pop(b * NT + q)
            rt = adj.tile([JB, TJB, BCOLS], F8, name="rt", tag="rt", bufs=2)
            nc.scalar.activation(rt[:, 0:ACT_SEGS, :], t[:, 0:ACT_SEGS, :],
                                 AF.Relu)
            nc.vector.tensor_scalar_max(rt[:, ACT_SEGS:TJB, :],
                                        t[:, ACT_SEGS:TJB, :], 0.0)
            hw8 = st["hw8"]
            psA = psum_blk("psA", b)
            psR = psum_blk("psR", b)
            for k in range(TJB // 2):
                jb = q * TJB + 2 * k
                stt = hw8[:, jb:jb + 2, :]
                first = (jb == 0)
                last = (jb == NJB - 2)
                nc.tensor.matmul(psA[:], stt, t[:, 2 * k:2 * k + 2, :],
                                 start=first, stop=last, perf_mode=DR)
                nc.tensor.matmul(psR[:], stt, rt[:, 2 * k:2 * k + 2, :],
                                 start=first, stop=last, perf_mode=DR)

        # --- per-block post work, emitted as thunks under later stream ---
        def cs_of(b):
            return slice(b * BCOLS, (b + 1) * BCOLS)

        def row_t(name, b, dt=F32):
            t = work.tile([1, BCOLS], dt, name=f"{name}{b}", tag=name, bufs=1)
            st[f"{name}{b}"] = t
            return t

        def th_xpxm(b):
            cs = cs_of(b)
            xp = work.tile([H, BCOLS], F32R, name=f"xp{b}", tag="xp", bufs=1)
            st[f"xp{b}"] = xp
            nc.scalar.copy(xp[:], psum_blk("psP", b)[:])
            xm = work.tile([H, BCOLS], F32R, name=f"xm{b}", tag="xm", bufs=1)
            st[f"xm{b}"] = xm
            # xm = (h - sum_h) + P   (== -M)
            nc.vector.scalar_tensor_tensor(
                xm[:], pf("hTf")[:, cs], pf("sumh"),
                xp[:].bitcast(F32), OP.subtract, OP.add)

        def th_scores(b):
            xp, xm = st[f"xp{b}"], st[f"xm{b}"]
            for col, nm in ((0, "ep"), (1, "em")):
                g = psG.tile([1, BCOLS], F32, name=f"{nm}{b}_ps", tag="g")
                nc.tensor.matmul(g[:], pr("vaP")[:, col:col + 1], xp[:],
                                 start=True, stop=False)
                nc.tensor.matmul(g[:], pr("vaM")[:, col:col + 1], xm[:],
                                 start=False, stop=True)
                dst = row_t(nm, b)
                nc.scalar.activation(dst[:], g[:], AF.Lrelu, alpha=ALPHA)

        def th_softmax1(b):
            ep, em = st[f"ep{b}"], st[f"em{b}"]
            m = row_t("mrow", b)
            nc.vector.tensor_tensor(m[:], ep[:], em[:], OP.max)
            wp = row_t("wp", b)
            nc.vector.tensor_tensor(wp[:], ep[:], m[:], OP.subtract)
            nc.scalar.activation(wp[:], wp[:], AF.Exp)
            wm = row_t("wm", b)
            nc.vector.tensor_tensor(wm[:], em[:], m[:], OP.subtract)
            nc.scalar.activation(wm[:], wm[:], AF.Exp)

        def th_softmax2(b):
            cs = cs_of(b)
            wp, wm = st[f"wp{b}"], st[f"wm{b}"]
            z = row_t("zrow", b)
            nc.vector.tensor_tensor(z[:], pf("cp")[:, cs], wp[:], OP.mult)
            tz = row_t("tz", b)
            nc.vector.tensor_tensor(tz[:], pf("cn")[:, cs], wm[:], OP.mult)
            nc.vector.tensor_tensor(z[:], z[:], tz[:], OP.add)
            # 1/z via exp(-ln z): ACT table accuracy is plenty for the
            # softmax normalizer, and it keeps the slow DVE reciprocal off
            # the critical path
            iz = row_t("iz", b)
            nc.scalar.activation(iz[:], z[:], AF.Ln)
            nc.scalar.activation(iz[:], iz[:], AF.Exp, scale=-1.0)
            ap = row_t("a_p", b, F32R)
            nc.vector.tensor_tensor(ap[:], wp[:], iz[:], OP.mult)
            am = row_t("a_m", b, F32R)
            nc.vector.tensor_tensor(am[:], wm[:], iz[:], OP.mult)
            # apm = ap - am, so es = apm*psR + am*psA (a*R + m*(A-R))
            apm = row_t("apm", b, F32R)
            nc.vector.tensor_tensor(apm[:], ap[:].bitcast(F32),
                                    am[:].bitcast(F32), OP.subtract)

        def bcast(src_r, b, name):
            g = psG.tile([H, BCOLS], F32, name=f"{name}{b}_ps", tag="g")
            nc.tensor.matmul(g[:], pr("ones1"), src_r, start=True, stop=True)
            t = work.tile([H, BCOLS], F32, name=f"{name}{b}", tag=name, bufs=1)
            st[f"{name}{b}"] = t
            nc.scalar.copy(t[:], g[:])
            return t

        def th_bcasts(b):
            cs = cs_of(b)
            bcast(st[f"apm{b}"][:], b, "apm_b")
            bcast(st[f"a_m{b}"][:], b, "am_b")
            bcast(pr("de")[:, cs], b, "de_b")
            bcast(pr("dn")[:, cs], b, "dn_b")

        # --- GRU pieces (transposed layout [feat, col]) ---
        def gru_gates(b, xs, w_rz, w_n, whh_rz, whh_hn, b_rz, b_n2, pref):
            hTr = pr("hTr")[:, cs_of(b)]
            g_rz = psG.tile([2 * H, BCOLS], F32, name=f"{pref}rz{b}", tag="g")
            mms = [(pr(w), mv) for mv, w in zip(xs, w_rz)]
            mms.append((pr(whh_rz), hTr))
            for i, (lh, mv) in enumerate(mms):
                nc.tensor.matmul(g_rz[:], lh, mv, start=(i == 0),
                                 stop=(i == len(mms) - 1))
            rz = work.tile([2 * H, BCOLS], F32, name=f"{pref}rzt{b}",
                           tag=f"{pref}rzt", bufs=1)
            st[f"{pref}rz{b}"] = rz
            nc.scalar.activation(rz[:], g_rz[:], AF.Sigmoid, bias=pf(b_rz))
            # unpack z to partition 0 (tensor_tensor requires same start
            # partition on all operands; a shifted COPY is legal)
            z_t = work.tile([H, BCOLS], F32, name=f"{pref}z{b}",
                            tag=f"g{pref}z", bufs=1)
            st[f"{pref}z{b}"] = z_t
            nc.gpsimd.tensor_copy(z_t[:], rz[H:2 * H, :])

            g_n = psG.tile([H, BCOLS], F32, name=f"{pref}n{b}", tag="g")
            for i, (mv, w) in enumerate(zip(xs, w_n)):
                nc.tensor.matmul(g_n[:], pr(w), mv, start=(i == 0),
                                 stop=(i == len(xs) - 1))
            ns = work.tile([H, BCOLS], F32, name=f"{pref}ns{b}",
                           tag=f"{pref}ns", bufs=1)
            st[f"{pref}ns{b}"] = ns
            nc.scalar.activation(ns[:], g_n[:], AF.Identity,
                                 bias=pf(b_n2)[:, 0:1])

            g_h = psG.tile([H, BCOLS], F32, name=f"{pref}hn{b}", tag="g")
            nc.tensor.matmul(g_h[:], pr(whh_hn), hTr, start=True, stop=True)
            hn = work.tile([H, BCOLS], F32, name=f"{pref}hnt{b}",
                           tag=f"{pref}hnt", bufs=1)
            st[f"{pref}hn{b}"] = hn
            nc.scalar.activation(hn[:], g_h[:], AF.Identity,
                                 bias=pf(b_n2)[:, 1:2])

        def gru_finish(b, pref, c0=0, c1=BCOLS):
            """n = tanh(ns + r*hn); out = n + z*(h - n), over cols [c0:c1)"""
            cs = slice(b * BCOLS + c0, b * BCOLS + c1)
            hsl = slice(c0, c1)
            rz, ns, hn = st[f"{pref}rz{b}"], st[f"{pref}ns{b}"], st[f"{pref}hn{b}"]
            key = f"{pref}o{b}"
            if key not in st:
                st[key] = work.tile([H, BCOLS], F32, name=key, tag=f"g{pref}o",
                                    bufs=1)
                st[f"{pref}ng{b}"] = work.tile([H, BCOLS], F32,
                                               name=f"{pref}ng{b}",
                                               tag=f"g{pref}n", bufs=1)
                st[f"{pref}dd{b}"] = work.tile([H, BCOLS], F32,
                                               name=f"{pref}dd{b}",
                                               tag=f"g{pref}d", bufs=1)
            og, n_g, d = st[key], st[f"{pref}ng{b}"], st[f"{pref}dd{b}"]
            nc.vector.tensor_tensor(d[:, hsl], hn[:, hsl], rz[0:H, hsl],
                                    OP.mult)
            nc.gpsimd.tensor_tensor(ns[:, hsl], ns[:, hsl], d[:, hsl], OP.add)
            nc.scalar.activation(n_g[:, hsl], ns[:, hsl], AF.Tanh)
            nc.vector.tensor_tensor(d[:, hsl], pf("hTf")[:, cs], n_g[:, hsl],
                                    OP.subtract)
            nc.vector.tensor_tensor(og[:, hsl], d[:, hsl],
                                    st[f"{pref}z{b}"][:, hsl], OP.mult)
            nc.gpsimd.tensor_tensor(og[:, hsl], og[:, hsl], n_g[:, hsl],
                                    OP.add)
            return og

        def th_egru_gates(b):
            gru_gates(b, [st[f"xp{b}"][:], st[f"xm{b}"][:]],
                      ["weP_rz", "weM_rz"], ["weP_n", "weM_n"],
                      "whe_rz", "whe_hn", "be_rz", "be_n", "e")

        def th_egru_finish(b):
            st[f"eo{b}"] = gru_finish(b, "e")

        def th_fin_e(b):
            fin = work.tile([H, BCOLS], F32, name=f"fin{b}", tag="fin", bufs=1)
            st[f"fin{b}"] = fin
            nc.vector.tensor_tensor(fin[:], st[f"de_b{b}"][:], st[f"eo{b}"][:],
                                    OP.mult)

        def th_es(b):
            # es = ap*R@hw + am*(A-R)@hw = apm*psR + am*psA (PSUM reads)
            t1 = work.tile([H, BCOLS], F32, name=f"t1{b}", tag="t1", bufs=1)
            nc.vector.tensor_tensor(t1[:], st[f"apm_b{b}"][:],
                                    psum_blk("psR", b)[:], OP.mult)
            t2 = work.tile([H, BCOLS], F32, name=f"t2{b}", tag="t2", bufs=1)
            nc.vector.tensor_tensor(t2[:], st[f"am_b{b}"][:],
                                    psum_blk("psA", b)[:], OP.mult)
            es = work.tile([H, BCOLS], F32R, name=f"es{b}", tag="es", bufs=1)
            st[f"es{b}"] = es
            nc.vector.tensor_tensor(es[:], t1[:], t2[:], OP.add)

        def th_ngru_gates(b):
            gru_gates(b, [st[f"es{b}"][:]], ["wn_rz"], ["wn_n"],
                      "whn_rz", "whn_hn", "bn_rz", "bn_n", "n")

        def th_combine(b, c0, c1):
            cs = slice(b * BCOLS + c0, b * BCOLS + c1)
            hsl = slice(c0, c1)
            no = gru_finish(b, "n", c0, c1)
            key = f"tf{b}"
            if key not in st:
                st[key] = work.tile([H, BCOLS], F32, name=key, tag="tf",
                                    bufs=1)
                st[f"ot{b}"] = work.tile([H, BCOLS], F32, name=f"ot{b}",
                                         tag="ot", bufs=1)
            tf, ot = st[key], st[f"ot{b}"]
            nc.vector.tensor_tensor(tf[:, hsl], st[f"dn_b{b}"][:, hsl],
                                    no[:, hsl], OP.mult)
            nc.gpsimd.tensor_tensor(ot[:, hsl], st[f"fin{b}"][:, hsl],
                                    tf[:, hsl], OP.add)
            nc.sync.dma_start(out[:, cs], ot[:, hsl])
            if DEBUG_DUMP and c1 == BCOLS:
                csf = cs_of(b)
                for nm, key2 in [("d_xp", f"xp{b}"), ("d_xm", f"xm{b}"),
                                 ("d_es", f"es{b}"), ("d_eo", f"eo{b}"),
                                 ("d_no", f"no{b}")]:
                    if key2 in st:
                        nc.sync.dma_start(dbg[nm][:, csf],
                                          st[key2][:].bitcast(F32))
                for nm, key2 in [("d_ep", f"ep{b}"), ("d_em", f"em{b}"),
                                 ("d_ap", f"a_p{b}"), ("d_am", f"a_m{b}")]:
                    nc.sync.dma_start(dbg[nm][:, csf],
                                      st[key2][:].bitcast(F32))

        # ================= schedule =================
        NSLOT = NBLK * 2 * NT
        sched = {}

        def at(slot, th):
            sched.setdefault(slot, []).append(th)

        for b in range(NBLK):
            s0 = b * 2 * NT
            at(s0 + NT - 1, lambda b=b: th_xpxm(b))
            at(s0 + NT + 0, lambda b=b: th_scores(b))
            at(s0 + NT + 1, lambda b=b: th_softmax1(b))
            at(s0 + NT + 2, lambda b=b: th_softmax2(b))
            at(s0 + NT + 3, lambda b=b: th_bcasts(b))
            at(s0 + NT + 4, lambda b=b: th_egru_gates(b))
            at(s0 + NT + 6, lambda b=b: th_egru_finish(b))
            at(s0 + NT + 7, lambda b=b: th_fin_e(b))
            at(s0 + 2 * NT + 0, lambda b=b: th_es(b))
            at(s0 + 2 * NT + 1, lambda b=b: th_ngru_gates(b))
            at(s0 + 2 * NT + 3, lambda b=b: th_combine(b, 0, 256))
            at(s0 + 2 * NT + 4, lambda b=b: th_combine(b, 256, BCOLS))

        # initial DMAs: first mask tile + first stationary quarter ASAP
        dma_mask(0)
        # pre-warm every ACT function table during the DMA dead time so no
        # 1.3us ACT_TABLE_LOAD lands mid-chain later
        wtab = work.tile([1, 8], F32, name="wtab", tag="wtab", bufs=1)
        nc.vector.memset(wtab[:], 0.0)
        for fn in (AF.Relu, AF.Lrelu, AF.Exp, AF.Sigmoid, AF.Tanh,
                   AF.Identity, AF.Ln):
            nc.scalar.activation(wtab[:], wtab[:], fn, alpha=ALPHA)
        nc.vector.memset(wtab[:], 0.0)
        QW = 16 * H
        for i in range(4):
            t = stat.tile([JB, QW], BF16, name=f"hb{i}")
            st[f"hb{i}"] = t
            nc.sync.dma_start(t[:], hb_d[:, i * QW:(i + 1) * QW])
            if i == 0:
                dma_mask(1)
        hw8_t = stat.tile([JB, NJB, H], F8, name="hw8")
        st["hw8"] = hw8_t
        nc.sync.dma_start(hw8_t[:], hw8_d[:])
        dma_edge(0)
        st["pr"] = stat.tile([JB, PR_COLS], F32R, name="pr")
        nc.sync.dma_start(st["pr"][:], pr_d[:])
        st["pf"] = stat.tile([JB, PF_COLS], F32, name="pf")
        nc.sync.dma_start(st["pf"][:], pf_d[:])

        mask_issued = 2
        edge_issued = 1

        for s in range(NSLOT):
            b, phase = divmod(s, 2 * NT)
            while mask_issued < NBLK * NT:
                g = mask_issued
                cons = (g // NT) * 2 * NT + (g % NT)
                if cons > s + 2:
                    break
                dma_mask(g)
                mask_issued += 1
            while edge_issued < NBLK * NT:
                g = edge_issued
                cons = (g // NT) * 2 * NT + NT + (g % NT)
                if cons > s + 2:
                    break
                dma_edge(g)
                edge_issued += 1
            if phase < NT:
                emit_mask_tile(b, phase)
            else:
                emit_edge_tile(b, phase - NT)
            for th in sched.pop(s, []):
                th()
        for s in sorted(sched):
            for th in sched[s]:
                th()

    _split_multiwaits(nc)
    return nc


def _host_prep(inputs):
    import ml_dtypes
    BF = ml_dtypes.bfloat16
    F8 = ml_dtypes.float8_e4m3

    h = np.ascontiguousarray(inputs["h"], dtype=np.float32)
    node_adj = inputs["node_adj"]
    edge_adj = inputs["edge_adj"]
    W_gat = np.asarray(inputs["W_gat"], dtype=np.float32)
    a_gat = np.asarray(inputs["a_gat"], dtype=np.float32)
    w_ih_e = np.asarray(inputs["w_ih_e"], dtype=np.float32)
    w_hh_e = np.asarray(inputs["w_hh_e"], dtype=np.float32)
    b_ih_e = np.asarray(inputs["b_ih_e"], dtype=np.float32)
    b_hh_e = np.asarray(inputs["b_hh_e"], dtype=np.float32)
    w_ih_n = np.asarray(inputs["w_ih_n"], dtype=np.float32)
    w_hh_n = np.asarray(inputs["w_hh_n"], dtype=np.float32)
    b_ih_n = np.asarray(inputs["b_ih_n"], dtype=np.float32)
    b_hh_n = np.asarray(inputs["b_hh_n"], dtype=np.float32)

    d_node = np.ascontiguousarray(np.diag(node_adj)).astype(np.float32)
    d_edge = np.ascontiguousarray(np.diag(edge_adj)).astype(np.float32)

    idx = np.arange(N)
    posn = (node_adj.T > 0).astype(F8)
    posn[idx, idx] = F8(0)
    eat = edge_adj.T.astype(F8)
    eat[idx, idx] = F8(0)
    cp_full = ((edge_adj > 0).sum(axis=1)
               - (np.diag(edge_adj) > 0)).astype(np.float32)
    cn_full = (N - 1) - cp_full

    hw = (h @ W_gat).astype(np.float32)
    hb = np.ascontiguousarray(
        h.reshape(NJB, JB, H).transpose(1, 0, 2).reshape(JB, NJB * H)
    ).astype(BF)
    # edge stationary, pair-interleaved for DoubleRow:
    # hw8[p, 2q+s, m] = hw[q*256 + s*128 + p, m]
    hw8 = np.ascontiguousarray(
        hw.reshape(NPAIR, 2, JB, H).transpose(2, 0, 1, 3).reshape(JB, NJB, H)
    ).astype(F8)
    sum_h = h.sum(axis=0, dtype=np.float64).astype(np.float32)

    a1 = a_gat[0:H, 0]
    a2 = a_gat[H:2 * H, 0]
    wih_eT = w_ih_e.T
    wihn_T = w_ih_n.T
    vals = {
        "weP_rz": wih_eT[0:H, 0:2 * H], "weM_rz": -wih_eT[H:2 * H, 0:2 * H],
        "whe_rz": w_hh_e.T[:, 0:2 * H], "wn_rz": wihn_T[:, 0:2 * H],
        "whn_rz": w_hh_n.T[:, 0:2 * H],
        "weP_n": wih_eT[0:H, 2 * H:], "weM_n": -wih_eT[H:2 * H, 2 * H:],
        "whe_hn": w_hh_e.T[:, 2 * H:], "wn_n": wihn_T[:, 2 * H:],
        "whn_hn": w_hh_n.T[:, 2 * H:],
        "vaP": np.stack([W_gat @ a1, W_gat @ a2], axis=1),
        "vaM": np.stack([-(W_gat @ a2), -(W_gat @ a1)], axis=1),
        "ones1": np.ones((1, H)),
        "be_rz": (b_ih_e + b_hh_e)[0:2 * H].reshape(2 * H, 1),
        "bn_rz": (b_ih_n + b_hh_n)[0:2 * H].reshape(2 * H, 1),
        "be_n": np.stack([b_ih_e[2 * H:], b_hh_e[2 * H:]], 1),
        "bn_n": np.stack([b_ih_n[2 * H:], b_hh_n[2 * H:]], 1),
        "sumh": sum_h.reshape(H, 1),
    }

    def pack(layout, per_core):
        cols = PR_COLS if layout is _PR else PF_COLS
        arr = np.zeros((JB, cols), np.float32)
        for nm, (p, c, w) in layout.items():
            v = per_core.get(nm)
            if v is None:
                v = vals[nm]
            arr[0:p, c:c + w] = v
        return arr

    def stream_pack(x):
        y = x.reshape(NT, TJB, JB, NBLK, BCOLS)
        return np.ascontiguousarray(
            y.transpose(3, 0, 2, 1, 4).reshape(NBLK * NT * JB, TJB, BCOLS))

    in_maps = []
    for c in range(NCORES):
        sl = slice(c * ROWS, (c + 1) * ROWS)
        hTc = np.ascontiguousarray(h[sl].T)
        per_r = {"dn": d_node[sl].reshape(1, ROWS),
                 "de": d_edge[sl].reshape(1, ROWS), "hTr": hTc}
        per_f = {"cp": cp_full[sl].reshape(1, ROWS),
                 "cn": cn_full[sl].reshape(1, ROWS), "hTf": hTc}
        m = {
            "msk": stream_pack(posn[:, sl]),
            "eat": stream_pack(eat[:, sl]),
            "hb": hb, "hw8": hw8,
            "pr": pack(_PR, per_r),
            "pf": pack(_PF, per_f),
        }
        in_maps.append(m)
    return in_maps


def _run(inputs, trace=False, tmpdir=None):
    from concourse.bass_utils import run_bass_kernel_spmd

    in_maps = _host_prep(inputs)
    nc = _build_nc()
    res = run_bass_kernel_spmd(nc, in_maps, core_ids=list(range(NCORES)),
                               trace=trace, tmpdir=tmpdir)
    outs = [res.results[c]["out"] for c in range(NCORES)]       # [64, 1024] each
    full = np.concatenate([o.T for o in outs], axis=0)          # [8192, 64]
    return np.ascontiguousarray(full, dtype=np.float32), res


def kernel(**inputs):
    out, _ = _run(inputs, trace=False)
    return out


# revision 13
# speedup vs baseline: 1.0376x; 1.0010x over previous
"""Trainium2 Bass kernel for nn_FactorGraphGRU (N=8192, H=64, 8 NeuronCores).

v3 strategy (memory regime): row-shard the output across 8 cores (1024
rows each), then split each core's rows into NBLK=2 column blocks of
512 and stream the contraction dim j column-block-major.  Per block:

  mask phase: 64 bf16 matmuls  psP_b += h(jb)^T-stationary @ posmask
  edge phase: 32+32 fp8 DoubleRow matmuls (K=256 per MM)
              psR_b += hw8 @ relu(eat),  psN_b += hw8 @ min(eat,0)

The relu/min split of the edge adjacency is done on the HOST (same
total DMA bytes as streaming eat + on-chip relu, but saves ~25us/block
of ACT+DVE work and a tile pool).  edge_support is then
es = ap*psR + am*psN with direct PSUM reads.

The per-row post-chain (attention scores, analytic softmax via
Ln/Exp reciprocal, edge GRU, es, node GRU, diag combine) for block b
is emitted under block b's edge phase / block b+1's mask phase, so
only the last block's es+nodeGRU+combine is exposed.

All small parameters ride in TWO packed block DMAs (one f32r for
matmul stationaries, one f32 for biases/rows) so the sync queue isn't
jammed by ~20 small descriptor issues.  The mask stationary h stays
bf16 (fp8 h measured 0.033 rel err, over the 2e-2 gate); the edge
stationary hw is fp8 (no measurable rel-err change).  M (negative
node support) comes from the no-exact-zeros complement
M = (sum_h - h_i) - P.
"""

import numpy as np
from contextlib import ExitStack

N = 8192
H = 64
NCORES = 8
ROWS = N // NCORES        # 1024 output rows per core
JB = 128                  # contraction block (SBUF partitions)
NJB = N // JB             # 64
NPAIR = NJB // 2          # 32
NBLK = 2                  # column blocks per core
BCOLS = ROWS // NBLK      # 512
TJB = 8                   # j-blocks per streamed tile
NT = NJB // TJB           # 8 stream tiles per (block, stream)
ALPHA = 0.2               # leaky relu slope
DEBUG_DUMP = False

# --- packed parameter layouts (shared between host prep and kernel) ---
# f32r pack [128, *]: matmul stationaries + f32r rows
_PR = {}
_c = 0
for _nm, _p, _w in [("weP_rz", 64, 128), ("weM_rz", 64, 128),
                    ("whe_rz", 64, 128), ("wn_rz", 64, 128),
                    ("whn_rz", 64, 128), ("weP_n", 64, 64),
                    ("weM_n", 64, 64), ("whe_hn", 64, 64),
                    ("wn_n", 64, 64), ("whn_hn", 64, 64),
                    ("vaP", 64, 2), ("vaM", 64, 2),
                    ("ones1", 1, 64), ("dn", 1, ROWS), ("de", 1, ROWS),
                    ("hTr", 64, ROWS)]:
    _PR[_nm] = (_p, _c, _w)
    _c += _w
PR_COLS = _c
# f32 pack [128, *]: biases, rows, h
_PF = {}
_c = 0
for _nm, _p, _w in [("be_rz", 128, 1), ("bn_rz", 128, 1),
                    ("be_n", 64, 2), ("bn_n", 64, 2), ("sumh", 64, 1),
                    ("cp", 1, ROWS), ("cn", 1, ROWS), ("hTf", 64, ROWS)]:
    _PF[_nm] = (_p, _c, _w)
    _c += _w
PF_COLS = _c


# ---------------------------------------------------------------------------
# walrus workaround: this toolchain accepts at most ONE sync wait per
# instruction; Tile attaches several.  Rewrite the BIR so every extra wait
# rides on its own NoOp carrier right before the instruction.
# ---------------------------------------------------------------------------
def _split_multiwaits(nc):
    import bass_rust
    import concourse.mybir as mybir

    ctr = [0]

    def carrier(engine, wait):
        ctr[0] += 1
        nop = bass_rust.InstNoOp(name=f"WS-{ctr[0]}", engine=engine, ins=[], outs=[])
        nop.sync_info = mybir.SyncInfo(on_wait=[wait], on_update=[])
        return nop

    for fn in nc.m.functions:
        stack = list(fn.blocks)
        while stack:
            bb = stack.pop()
            stack.extend(getattr(bb, "blocks", []) or [])
            out = []
            changed = False
            for inst in bb.instructions:
                si = inst.sync_info
                waits = list(si.on_wait) if si is not None and si.on_wait else []
                if len(waits) > 1:
                    for w in waits[:-1]:
                        out.append(carrier(inst.engine, w))
                    si.on_wait = [waits[-1]]
                    changed = True
                out.append(inst)
            if changed:
                bb.instructions = out


def _build_nc():
    import concourse.bass as bass
    import concourse.tile as tile
    from concourse import mybir

    F32 = mybir.dt.float32
    F32R = mybir.dt.float32r
    BF16 = mybir.dt.bfloat16
    F8 = mybir.dt.float8e4
    AF = mybir.ActivationFunctionType
    OP = mybir.AluOpType
    DR = mybir.MatmulPerfMode.DoubleRow

    nc = bass.Bass("TRN2", target_bir_lowering=False, debug=False,
                   num_devices=NCORES)

    msk_d = nc.dram_tensor("msk", [NBLK * NT * JB, TJB, BCOLS], F8,
                           kind="ExternalInput").ap()
    eat_d = nc.dram_tensor("eat", [NT * JB, TJB, BCOLS], F8,
                           kind="ExternalInput").ap()
    eatR_d = nc.dram_tensor("eatR", [NT * JB, TJB, BCOLS], F8,
                            kind="ExternalInput").ap()
    eatN_d = nc.dram_tensor("eatN", [NT * JB, TJB, BCOLS], F8,
                            kind="ExternalInput").ap()
    hb_d = nc.dram_tensor("hb", [JB, NJB * H], BF16, kind="ExternalInput").ap()
    hw8_d = nc.dram_tensor("hw8", [JB, NJB, H], F8, kind="ExternalInput").ap()
    pr_d = nc.dram_tensor("pr", [JB, PR_COLS], F32R, kind="ExternalInput").ap()
    pf_d = nc.dram_tensor("pf", [JB, PF_COLS], F32, kind="ExternalInput").ap()
    out = nc.dram_tensor("out", [H, ROWS], F32, kind="ExternalOutput").ap()
    dbg = {}
    if DEBUG_DUMP:
        for nm, sh in [("d_xp", [H, ROWS]), ("d_xm", [H, ROWS]),
                       ("d_ep", [1, ROWS]), ("d_em", [1, ROWS]),
                       ("d_ap", [1, ROWS]), ("d_am", [1, ROWS]),
                       ("d_es", [H, ROWS]), ("d_eo", [H, ROWS]),
                       ("d_no", [H, ROWS])]:
            dbg[nm] = nc.dram_tensor(nm, sh, F32, kind="ExternalOutput").ap()

    with tile.TileContext(nc) as tc, ExitStack() as ctx:
        stat = ctx.enter_context(tc.tile_pool(name="stat", bufs=1))
        adj = ctx.enter_context(tc.tile_pool(name="adj", bufs=1))
        work = ctx.enter_context(tc.tile_pool(name="work", bufs=1))
        psS = ctx.enter_context(tc.tile_pool(name="psS", bufs=1, space="PSUM"))
        psG = ctx.enter_context(tc.tile_pool(name="psG", bufs=2, space="PSUM"))

        st = {}

        def pr(nm):
            p, c, w = _PR[nm]
            return st["pr"][0:p, c:c + w]

        def pf(nm):
            p, c, w = _PF[nm]
            return st["pf"][0:p, c:c + w]

        # --- streamed tiles ---
        mskt_tiles = {}
        et_tiles = {}
        etR_tiles = {}
        etN_tiles = {}

        def dma_mask(g):
            t = adj.tile([JB, TJB, BCOLS], F8, name="mskt", tag="mskt", bufs=3)
            nc.sync.dma_start(t[:], msk_d[g * JB:(g + 1) * JB, :, :])
            mskt_tiles[g] = t

        def dma_edge(g):
            # edge streams ride the gpsimd DMA queue so a ring-buffer wait
            # can never head-of-line-block the mask stream (sync queue).
            # block 0: raw eat (relu on-chip, DMA-light while DMA is hot);
            # block 1: host-split R/N (2x bytes but relu-free, and DMA is
            # idle by then -- frees ACT/DVE for the exposed tail)
            if g < NT:
                t = adj.tile([JB, TJB, BCOLS], F8, name="et", tag="et", bufs=3)
                nc.gpsimd.dma_start(t[:], eat_d[g * JB:(g + 1) * JB, :, :])
                et_tiles[g] = t
            else:
                r0 = (g - NT) * JB
                t = adj.tile([JB, TJB, BCOLS], F8, name="etR", tag="etR",
                             bufs=2)
                nc.gpsimd.dma_start(t[:], eatR_d[r0:r0 + JB, :, :])
                etR_tiles[g] = t
                t = adj.tile([JB, TJB, BCOLS], F8, name="etN", tag="etN",
                             bufs=2)
                nc.gpsimd.dma_start(t[:], eatN_d[r0:r0 + JB, :, :])
                etN_tiles[g] = t

        # --- accumulators: 2 bufs per tag + psG(2) = 8 PSUM banks ---
        def psum_blk(tag, b):
            key = f"{tag}{b}"
            if key not in st:
                st[key] = psS.tile([H, BCOLS], F32, name=key, tag=tag, bufs=2)
            return st[key]

        def emit_mask_tile(b, q):
            t = mskt_tiles.pop(b * NT + q)
            psP = psum_blk("psP", b)
            for s in range(TJB):
                jb = q * TJB + s
                hbt = st[f"hb{jb // 16}"]
                nc.tensor.matmul(psP[:], hbt[:, (jb % 16) * H:(jb % 16 + 1) * H],
                                 t[:, s, :],
                                 start=(jb == 0), stop=(jb == NJB - 1))

        ACT_SEGS = 2   # relu segments on ACT; rest on DVE (faster for fp8)

        def emit_edge_tile(b, q):
            g = b * NT + q
            if b == 0:
                t = et_tiles.pop(g)
                rt = adj.tile([JB, TJB, BCOLS], F8, name="rt", tag="rt",
                              bufs=2)
                nc.scalar.activation(rt[:, 0:ACT_SEGS, :], t[:, 0:ACT_SEGS, :],
                                     AF.Relu)
                nc.vector.tensor_scalar_max(rt[:, ACT_SEGS:TJB, :],
                                            t[:, ACT_SEGS:TJB, :], 0.0)
                tA, tR = t, rt
            else:
                tR = etR_tiles.pop(g)
                tN = etN_tiles.pop(g)
            hw8 = st["hw8"]
            psA = psum_blk("psA", b)
            psR = psum_blk("psR", b)
            for k in range(TJB // 2):
                jb = q * TJB + 2 * k
                stt = hw8[:, jb:jb + 2, :]
                first = (jb == 0)
                last = (jb == NJB - 2)
                if b == 0:
                    nc.tensor.matmul(psA[:], stt, tA[:, 2 * k:2 * k + 2, :],
                                     start=first, stop=last, perf_mode=DR)
                    nc.tensor.matmul(psR[:], stt, tR[:, 2 * k:2 * k + 2, :],
                                     start=first, stop=last, perf_mode=DR)
                else:
                    # psA accumulates A = R + N here (R first, N second)
                    nc.tensor.matmul(psA[:], stt, tR[:, 2 * k:2 * k + 2, :],
                                     start=first, stop=False, perf_mode=DR)
                    nc.tensor.matmul(psA[:], stt, tN[:, 2 * k:2 * k + 2, :],
                                     start=False, stop=last, perf_mode=DR)
                    nc.tensor.matmul(psR[:], stt, tR[:, 2 * k:2 * k + 2, :],
                                     start=first, stop=last, perf_mode=DR)

        # --- per-block post work, emitted as thunks under later stream ---
        def cs_of(b):
            return slice(b * BCOLS, (b + 1) * BCOLS)

        def row_t(name, b, dt=F32):
            t = work.tile([1, BCOLS], dt, name=f"{name}{b}", tag=name, bufs=1)
            st[f"{name}{b}"] = t
            return t

        def th_xpxm(b):
            cs = cs_of(b)
            xp = work.tile([H, BCOLS], F32R, name=f"xp{b}", tag="xp", bufs=1)
            st[f"xp{b}"] = xp
            nc.scalar.copy(xp[:], psum_blk("psP", b)[:])
            xm = work.tile([H, BCOLS], F32R, name=f"xm{b}", tag="xm", bufs=1)
            st[f"xm{b}"] = xm
            # xm = (h - sum_h) + P   (== -M)
            nc.vector.scalar_tensor_tensor(
                xm[:], pf("hTf")[:, cs], pf("sumh"),
                xp[:].bitcast(F32), OP.subtract, OP.add)

        def th_scores(b):
            xp, xm = st[f"xp{b}"], st[f"xm{b}"]
            for col, nm in ((0, "ep"), (1, "em")):
                g = psG.tile([1, BCOLS], F32, name=f"{nm}{b}_ps", tag="g")
                nc.tensor.matmul(g[:], pr("vaP")[:, col:col + 1], xp[:],
                                 start=True, stop=False)
                nc.tensor.matmul(g[:], pr("vaM")[:, col:col + 1], xm[:],
                                 start=False, stop=True)
                dst = row_t(nm, b)
                nc.scalar.activation(dst[:], g[:], AF.Lrelu, alpha=ALPHA)

        def th_softmax1(b):
            ep, em = st[f"ep{b}"], st[f"em{b}"]
            m = row_t("mrow", b)
            nc.vector.tensor_tensor(m[:], ep[:], em[:], OP.max)
            wp = row_t("wp", b)
            nc.vector.tensor_tensor(wp[:], ep[:], m[:], OP.subtract)
            nc.scalar.activation(wp[:], wp[:], AF.Exp)
            wm = row_t("wm", b)
            nc.vector.tensor_tensor(wm[:], em[:], m[:], OP.subtract)
            nc.scalar.activation(wm[:], wm[:], AF.Exp)

        def th_softmax2(b):
            cs = cs_of(b)
            wp, wm = st[f"wp{b}"], st[f"wm{b}"]
            z = row_t("zrow", b)
            nc.vector.tensor_tensor(z[:], pf("cp")[:, cs], wp[:], OP.mult)
            tz = row_t("tz", b)
            nc.vector.tensor_tensor(tz[:], pf("cn")[:, cs], wm[:], OP.mult)
            nc.vector.tensor_tensor(z[:], z[:], tz[:], OP.add)
            # 1/z via exp(-ln z): ACT table accuracy is plenty for the
            # softmax normalizer, and it keeps the slow DVE reciprocal off
            # the critical path
            iz = row_t("iz", b)
            nc.scalar.activation(iz[:], z[:], AF.Ln)
            nc.scalar.activation(iz[:], iz[:], AF.Exp, scale=-1.0)
            ap = row_t("a_p", b, F32R)
            nc.vector.tensor_tensor(ap[:], wp[:], iz[:], OP.mult)
            am = row_t("a_m", b, F32R)
            nc.vector.tensor_tensor(am[:], wm[:], iz[:], OP.mult)
            # apm = ap - am, so es = apm*psR + am*psA (a*R + m*(A-R))
            apm = row_t("apm", b, F32R)
            nc.vector.tensor_tensor(apm[:], ap[:].bitcast(F32),
                                    am[:].bitcast(F32), OP.subtract)

        def bcast(src_r, b, name, eng="act"):
            g = psG.tile([H, BCOLS], F32, name=f"{name}{b}_ps", tag="g")
            nc.tensor.matmul(g[:], pr("ones1"), src_r, start=True, stop=True)
            t = work.tile([H, BCOLS], F32, name=f"{name}{b}", tag=name, bufs=1)
            st[f"{name}{b}"] = t
            if eng == "act":
                nc.scalar.copy(t[:], g[:])
            else:
                nc.vector.tensor_copy(t[:], g[:])
            return t

        def th_bcasts(b):
            cs = cs_of(b)
            bcast(st[f"apm{b}"][:], b, "apm_b")
            bcast(st[f"a_m{b}"][:], b, "am_b")
            bcast(pr("de")[:, cs], b, "de_b", "dve")
            bcast(pr("dn")[:, cs], b, "dn_b", "dve")

        # --- GRU pieces (transposed layout [feat, col]) ---
        def gru_gates(b, xs, w_rz, w_n, whh_rz, whh_hn, b_rz, b_n2, pref):
            hTr = pr("hTr")[:, cs_of(b)]
            g_rz = psG.tile([2 * H, BCOLS], F32, name=f"{pref}rz{b}", tag="g")
            mms = [(pr(w), mv) for mv, w in zip(xs, w_rz)]
            mms.append((pr(whh_rz), hTr))
            for i, (lh, mv) in enumerate(mms):
                nc.tensor.matmul(g_rz[:], lh, mv, start=(i == 0),
                                 stop=(i == len(mms) - 1))
            rz = work.tile([2 * H, BCOLS], F32, name=f"{pref}rzt{b}",
                           tag=f"{pref}rzt", bufs=1)
            st[f"{pref}rz{b}"] = rz
            nc.scalar.activation(rz[:], g_rz[:], AF.Sigmoid, bias=pf(b_rz))
            # unpack z to partition 0 (tensor_tensor requires same start
            # partition on all operands; a shifted COPY is legal)
            z_t = work.tile([H, BCOLS], F32, name=f"{pref}z{b}",
                            tag=f"g{pref}z", bufs=1)
            st[f"{pref}z{b}"] = z_t
            nc.vector.tensor_copy(z_t[:], rz[H:2 * H, :])

            g_n = psG.tile([H, BCOLS], F32, name=f"{pref}n{b}", tag="g")
            for i, (mv, w) in enumerate(zip(xs, w_n)):
                nc.tensor.matmul(g_n[:], pr(w), mv, start=(i == 0),
                                 stop=(i == len(xs) - 1))
            ns = work.tile([H, BCOLS], F32, name=f"{pref}ns{b}",
                           tag=f"{pref}ns", bufs=1)
            st[f"{pref}ns{b}"] = ns
            nc.scalar.activation(ns[:], g_n[:], AF.Identity,
                                 bias=pf(b_n2)[:, 0:1])

            g_h = psG.tile([H, BCOLS], F32, name=f"{pref}hn{b}", tag="g")
            nc.tensor.matmul(g_h[:], pr(whh_hn), hTr, start=True, stop=True)
            hn = work.tile([H, BCOLS], F32, name=f"{pref}hnt{b}",
                           tag=f"{pref}hnt", bufs=1)
            st[f"{pref}hn{b}"] = hn
            nc.scalar.activation(hn[:], g_h[:], AF.Identity,
                                 bias=pf(b_n2)[:, 1:2])

        def gru_finish(b, pref, c0=0, c1=BCOLS):
            """n = tanh(ns + r*hn); out = n + z*(h - n), over cols [c0:c1)"""
            cs = slice(b * BCOLS + c0, b * BCOLS + c1)
            hsl = slice(c0, c1)
            rz, ns, hn = st[f"{pref}rz{b}"], st[f"{pref}ns{b}"], st[f"{pref}hn{b}"]
            key = f"{pref}o{b}"
            if key not in st:
                st[key] = work.tile([H, BCOLS], F32, name=key, tag=f"g{pref}o",
                                    bufs=1)
                st[f"{pref}ng{b}"] = work.tile([H, BCOLS], F32,
                                               name=f"{pref}ng{b}",
                                               tag=f"g{pref}n", bufs=1)
                st[f"{pref}dd{b}"] = work.tile([H, BCOLS], F32,
                                               name=f"{pref}dd{b}",
                                               tag=f"g{pref}d", bufs=1)
            og, n_g, d = st[key], st[f"{pref}ng{b}"], st[f"{pref}dd{b}"]
            nc.vector.tensor_tensor(d[:, hsl], hn[:, hsl], rz[0:H, hsl],
                                    OP.mult)
            nc.gpsimd.tensor_tensor(ns[:, hsl], ns[:, hsl], d[:, hsl], OP.add)
            nc.scalar.activation(n_g[:, hsl], ns[:, hsl], AF.Tanh)
            nc.vector.tensor_tensor(d[:, hsl], pf("hTf")[:, cs], n_g[:, hsl],
                                    OP.subtract)
            nc.vector.tensor_tensor(og[:, hsl], d[:, hsl],
                                    st[f"{pref}z{b}"][:, hsl], OP.mult)
            nc.gpsimd.tensor_tensor(og[:, hsl], og[:, hsl], n_g[:, hsl],
                                    OP.add)
            return og

        def th_egru_gates(b):
            gru_gates(b, [st[f"xp{b}"][:], st[f"xm{b}"][:]],
                      ["weP_rz", "weM_rz"], ["weP_n", "weM_n"],
                      "whe_rz", "whe_hn", "be_rz", "be_n", "e")

        def th_egru_finish(b):
            st[f"eo{b}"] = gru_finish(b, "e")

        def th_fin_e(b):
            fin = work.tile([H, BCOLS], F32, name=f"fin{b}", tag="fin", bufs=1)
            st[f"fin{b}"] = fin
            nc.vector.tensor_tensor(fin[:], st[f"de_b{b}"][:], st[f"eo{b}"][:],
                                    OP.mult)

        def th_es(b):
            # es = ap*R@hw + am*(A-R)@hw = apm*psR + am*psA (PSUM reads)
            t1 = work.tile([H, BCOLS], F32, name=f"t1{b}", tag="t1", bufs=1)
            nc.vector.tensor_tensor(t1[:], st[f"apm_b{b}"][:],
                                    psum_blk("psR", b)[:], OP.mult)
            t2 = work.tile([H, BCOLS], F32, name=f"t2{b}", tag="t2", bufs=1)
            nc.vector.tensor_tensor(t2[:], st[f"am_b{b}"][:],
                                    psum_blk("psA", b)[:], OP.mult)
            es = work.tile([H, BCOLS], F32R, name=f"es{b}", tag="es", bufs=1)
            st[f"es{b}"] = es
            nc.vector.tensor_tensor(es[:], t1[:], t2[:], OP.add)

        def th_ngru_gates(b):
            gru_gates(b, [st[f"es{b}"][:]], ["wn_rz"], ["wn_n"],
                      "whn_rz", "whn_hn", "bn_rz", "bn_n", "n")

        def th_combine(b, c0, c1):
            cs = slice(b * BCOLS + c0, b * BCOLS + c1)
            hsl = slice(c0, c1)
            no = gru_finish(b, "n", c0, c1)
            key = f"tf{b}"
            if key not in st:
                st[key] = work.tile([H, BCOLS], F32, name=key, tag="tf",
                                    bufs=1)
                st[f"ot{b}"] = work.tile([H, BCOLS], F32, name=f"ot{b}",
                                         tag="ot", bufs=1)
            tf, ot = st[key], st[f"ot{b}"]
            nc.vector.tensor_tensor(tf[:, hsl], st[f"dn_b{b}"][:, hsl],
                                    no[:, hsl], OP.mult)
            nc.gpsimd.tensor_tensor(ot[:, hsl], st[f"fin{b}"][:, hsl],
                                    tf[:, hsl], OP.add)
            nc.sync.dma_start(out[:, cs], ot[:, hsl])
            if DEBUG_DUMP and c1 == BCOLS:
                csf = cs_of(b)
                for nm, key2 in [("d_xp", f"xp{b}"), ("d_xm", f"xm{b}"),
                                 ("d_es", f"es{b}"), ("d_eo", f"eo{b}"),
                                 ("d_no", f"no{b}")]:
                    if key2 in st:
                        nc.sync.dma_start(dbg[nm][:, csf],
                                          st[key2][:].bitcast(F32))
                for nm, key2 in [("d_ep", f"ep{b}"), ("d_em", f"em{b}"),
                                 ("d_ap", f"a_p{b}"), ("d_am", f"a_m{b}")]:
                    nc.sync.dma_start(dbg[nm][:, csf],
                                      st[key2][:].bitcast(F32))

        # ================= schedule =================
        NSLOT = NBLK * 2 * NT
        sched = {}

        def at(slot, th):
            sched.setdefault(slot, []).append(th)

        # block 0: scores early, heavy chain under block-1 MASK phase
        # (block-0 edge phase is relu-heavy on ACT/DVE); block 1: chain
        # under its own relu-free edge phase, tail = es+nGRU+combine
        at(NT - 1, lambda: th_xpxm(0))
        at(NT + 0, lambda: th_scores(0))
        at(2 * NT + 0, lambda: th_softmax1(0))
        at(2 * NT + 1, lambda: th_softmax2(0))
        at(2 * NT + 2, lambda: th_bcasts(0))
        at(2 * NT + 3, lambda: th_egru_gates(0))
        at(2 * NT + 4, lambda: th_egru_finish(0))
        at(2 * NT + 5, lambda: th_fin_e(0))
        at(2 * NT + 5, lambda: th_es(0))
        at(2 * NT + 6, lambda: th_ngru_gates(0))
        at(2 * NT + 7, lambda: th_combine(0, 0, 256))
        at(2 * NT + 7, lambda: th_combine(0, 256, BCOLS))
        at(3 * NT - 1, lambda: th_xpxm(1))
        at(3 * NT + 0, lambda: th_scores(1))
        at(3 * NT + 1, lambda: th_softmax1(1))
        at(3 * NT + 2, lambda: th_softmax2(1))
        at(3 * NT + 3, lambda: th_bcasts(1))
        at(3 * NT + 4, lambda: th_egru_gates(1))
        at(3 * NT + 6, lambda: th_egru_finish(1))
        at(3 * NT + 7, lambda: th_fin_e(1))
        at(4 * NT + 0, lambda: th_es(1))
        at(4 * NT + 1, lambda: th_ngru_gates(1))
        at(4 * NT + 2, lambda: th_combine(1, 0, 256))
        at(4 * NT + 3, lambda: th_combine(1, 256, BCOLS))

        # initial DMAs: first mask tile + first stationary quarter ASAP
        dma_mask(0)
        # pre-warm every ACT function table during the DMA dead time so no
        # 1.3us ACT_TABLE_LOAD lands mid-chain later
        wtab = work.tile([1, 8], F32, name="wtab", tag="wtab", bufs=1)
        nc.vector.memset(wtab[:], 0.0)
        for fn in (AF.Relu, AF.Lrelu, AF.Exp, AF.Sigmoid, AF.Tanh,
                   AF.Identity, AF.Ln):
            nc.scalar.activation(wtab[:], wtab[:], fn, alpha=ALPHA)
        nc.vector.memset(wtab[:], 0.0)
        QW = 16 * H
        for i in range(4):
            t = stat.tile([JB, QW], BF16, name=f"hb{i}")
            st[f"hb{i}"] = t
            nc.sync.dma_start(t[:], hb_d[:, i * QW:(i + 1) * QW])
            if i == 0:
                dma_mask(1)
        # big param packs ride the gpsimd queue: the sync queue must stay
        # clear for the mask stream (pr/pf ahead of mask tiles starved the
        # tensor engine for ~10us in v4)
        dma_edge(0)
        hw8_t = stat.tile([JB, NJB, H], F8, name="hw8")
        st["hw8"] = hw8_t
        nc.gpsimd.dma_start(hw8_t[:], hw8_d[:])
        st["pf"] = stat.tile([JB, PF_COLS], F32, name="pf")
        nc.gpsimd.dma_start(st["pf"][:], pf_d[:])
        st["pr"] = stat.tile([JB, PR_COLS], F32R, name="pr")
        nc.gpsimd.dma_start(st["pr"][:], pr_d[:])

        mask_issued = 2
        edge_issued = 1

        for s in range(NSLOT):
            b, phase = divmod(s, 2 * NT)
            while mask_issued < NBLK * NT:
                g = mask_issued
                cons = (g // NT) * 2 * NT + (g % NT)
                if cons > s + 2:
                    break
                dma_mask(g)
                mask_issued += 1
            while edge_issued < NBLK * NT:
                g = edge_issued
                cons = (g // NT) * 2 * NT + NT + (g % NT)
                if cons > s + 2:
                    break
                dma_edge(g)
                edge_issued += 1
            if phase < NT:
                emit_mask_tile(b, phase)
            else:
                emit_edge_tile(b, phase - NT)
            for th in sched.pop(s, []):
                th()
        for s in sorted(sched):
            for th in sched[s]:
                th()

    _split_multiwaits(nc)
    return nc


def _host_prep(inputs):
    import ml_dtypes
    BF = ml_dtypes.bfloat16
    F8 = ml_dtypes.float8_e4m3

    h = np.ascontiguousarray(inputs["h"], dtype=np.float32)
    node_adj = inputs["node_adj"]
    edge_adj = inputs["edge_adj"]
    W_gat = np.asarray(inputs["W_gat"], dtype=np.float32)
    a_gat = np.asarray(inputs["a_gat"], dtype=np.float32)
    w_ih_e = np.asarray(inputs["w_ih_e"], dtype=np.float32)
    w_hh_e = np.asarray(inputs["w_hh_e"], dtype=np.float32)
    b_ih_e = np.asarray(inputs["b_ih_e"], dtype=np.float32)
    b_hh_e = np.asarray(inputs["b_hh_e"], dtype=np.float32)
    w_ih_n = np.asarray(inputs["w_ih_n"], dtype=np.float32)
    w_hh_n = np.asarray(inputs["w_hh_n"], dtype=np.float32)
    b_ih_n = np.asarray(inputs["b_ih_n"], dtype=np.float32)
    b_hh_n = np.asarray(inputs["b_hh_n"], dtype=np.float32)

    d_node = np.ascontiguousarray(np.diag(node_adj)).astype(np.float32)
    d_edge = np.ascontiguousarray(np.diag(edge_adj)).astype(np.float32)

    idx = np.arange(N)
    posn = (node_adj.T > 0).astype(F8)
    posn[idx, idx] = F8(0)
    eat = edge_adj.T.astype(F8).astype(np.float32)
    eat[idx, idx] = 0.0
    eatR = np.maximum(eat, 0.0).astype(F8)
    eatN = np.minimum(eat, 0.0).astype(F8)
    eat = eat.astype(F8)
    cp_full = ((edge_adj > 0).sum(axis=1)
               - (np.diag(edge_adj) > 0)).astype(np.float32)
    cn_full = (N - 1) - cp_full

    hw = (h @ W_gat).astype(np.float32)
    hb = np.ascontiguousarray(
        h.reshape(NJB, JB, H).transpose(1, 0, 2).reshape(JB, NJB * H)
    ).astype(BF)
    # edge stationary, pair-interleaved for DoubleRow:
    # hw8[p, 2q+s, m] = hw[q*256 + s*128 + p, m]
    hw8 = np.ascontiguousarray(
        hw.reshape(NPAIR, 2, JB, H).transpose(2, 0, 1, 3).reshape(JB, NJB, H)
    ).astype(F8)
    sum_h = h.sum(axis=0, dtype=np.float64).astype(np.float32)

    a1 = a_gat[0:H, 0]
    a2 = a_gat[H:2 * H, 0]
    wih_eT = w_ih_e.T
    wihn_T = w_ih_n.T
    vals = {
        "weP_rz": wih_eT[0:H, 0:2 * H], "weM_rz": -wih_eT[H:2 * H, 0:2 * H],
        "whe_rz": w_hh_e.T[:, 0:2 * H], "wn_rz": wihn_T[:, 0:2 * H],
        "whn_rz": w_hh_n.T[:, 0:2 * H],
        "weP_n": wih_eT[0:H, 2 * H:], "weM_n": -wih_eT[H:2 * H, 2 * H:],
        "whe_hn": w_hh_e.T[:, 2 * H:], "wn_n": wihn_T[:, 2 * H:],
        "whn_hn": w_hh_n.T[:, 2 * H:],
        "vaP": np.stack([W_gat @ a1, W_gat @ a2], axis=1),
        "vaM": np.stack([-(W_gat @ a2), -(W_gat @ a1)], axis=1),
        "ones1": np.ones((1, H)),
        "be_rz": (b_ih_e + b_hh_e)[0:2 * H].reshape(2 * H, 1),
        "bn_rz": (b_ih_n + b_hh_n)[0:2 * H].reshape(2 * H, 1),
        "be_n": np.stack([b_ih_e[2 * H:], b_hh_e[2 * H:]], 1),
        "bn_n": np.stack([b_ih_n[2 * H:], b_hh_n[2 * H:]], 1),
        "sumh": sum_h.reshape(H, 1),
    }

    def pack(layout, per_core):
        cols = PR_COLS if layout is _PR else PF_COLS
        arr = np.zeros((JB, cols), np.float32)
        for nm, (p, c, w) in layout.items():
            v = per_core.get(nm)
            if v is None:
                v = vals[nm]
            arr[0:p, c:c + w] = v
        return arr

    def stream_pack(x):
        y = x.reshape(NT, TJB, JB, NBLK, BCOLS)
        return np.ascontiguousarray(
            y.transpose(3, 0, 2, 1, 4).reshape(NBLK * NT * JB, TJB, BCOLS))

    def stream_pack1(x, blk):
        # one block's columns only: [N, BCOLS] -> [NT*JB, TJB, BCOLS]
        y = x.reshape(NT, TJB, JB, ROWS)[:, :, :,
                                         blk * BCOLS:(blk + 1) * BCOLS]
        return np.ascontiguousarray(
            y.transpose(0, 2, 1, 3).reshape(NT * JB, TJB, BCOLS))

    in_maps = []
    for c in range(NCORES):
        sl = slice(c * ROWS, (c + 1) * ROWS)
        hTc = np.ascontiguousarray(h[sl].T)
        per_r = {"dn": d_node[sl].reshape(1, ROWS),
                 "de": d_edge[sl].reshape(1, ROWS), "hTr": hTc}
        per_f = {"cp": cp_full[sl].reshape(1, ROWS),
                 "cn": cn_full[sl].reshape(1, ROWS), "hTf": hTc}
        m = {
            "msk": stream_pack(posn[:, sl]),
            "eat": stream_pack1(eat[:, sl], 0),
            "eatR": stream_pack1(eatR[:, sl], 1),
            "eatN": stream_pack1(eatN[:, sl], 1),
            "hb": hb, "hw8": hw8,
            "pr": pack(_PR, per_r),
            "pf": pack(_PF, per_f),
        }
        in_maps.append(m)
    return in_maps


def _run(inputs, trace=False, tmpdir=None):
    from concourse.bass_utils import run_bass_kernel_spmd

    in_maps = _host_prep(inputs)
    nc = _build_nc()
    res = run_bass_kernel_spmd(nc, in_maps, core_ids=list(range(NCORES)),
                               trace=trace, tmpdir=tmpdir)
    outs = [res.results[c]["out"] for c in range(NCORES)]       # [64, 1024] each
    full = np.concatenate([o.T for o in outs], axis=0)          # [8192, 64]
    return np.ascontiguousarray(full, dtype=np.float32), res


def kernel(**inputs):
    out, _ = _run(inputs, trace=False)
    return out
